# revision 30
# baseline (speedup 1.0000x reference)
"""Cross-graph attention (block-diagonal segment-local attention) on 8 trn2 cores.

Graphs (batch ids) are contiguous segments in the sorted atom_batch /
residue_batch arrays; attention is block-diagonal per graph.  32 graphs are
assigned 4-per-core to 8 cores, one graph per "slot".  Slot shapes are
variable per rank (slot i on every core has the same (atom-chunks,
residue-chunks, atom-width) capacity = max over the 8 graphs assigned to
that rank), so all 8 cores run one identical SPMD program with minimal
padding.

All matmuls run in bf16 (rate 1 cycle/row at ANY moving width on TRN2, vs
f32r needing >=256), with fp32 PSUM accumulation.  Tolerance is 2e-2; bf16
keeps us well inside it.

Per-core program (everything transposed; no on-device transposes anywhere):
  - CT = M' @ R^T per slot, where M' = (W_q^T W_k)/sqrt(DH) is folded
    host-side from the parameters: kills the separate Q projection.
  - V_k = R_k @ W_v^T per residue chunk ([residues, feats] layout).
  - S^T chunk = CT_k^T @ A^T  ->  exp via one ACT instruction per (slot,
    residue-chunk).  No mask bias needed: padded residues have resT = 0 so
    CT col = 0, exp(0) = 1, but V row = 0 (no numerator pollution) and
    valid = 0 (no denominator pollution).
  - U^T = sum_k V_k^T @ ES_k  (feats x atoms, accumulated in PSUM).
  - denom = sum_k ES_k^T @ valid_k : single-column matmuls, ~free.
  - normalization + residual add run host-side:
      out = atom_h + (U^T / denom)^T.

Scheduling: HWDGE charges ~625ns per DMA (serialized), so ALL inputs live
in one packed dram tensor loaded with 4 per-slot DMAs, and the denominator
rides in the tail of the single output tensor (bf16).  S(g+1) is emitted
before U(g) so the PE never idles while ACT computes exp; a dummy exp at
t=0 hides the activation-table load under the input DMAs.  GPSIMD cannot
touch PSUM, so PSUM->SBUF copies run on DVE (ACT is saturated by exp).
"""

import sys

if "/opt/trn_rl_repo" not in sys.path:
    sys.path.insert(0, "/opt/trn_rl_repo")

import numpy as np
import ml_dtypes

import concourse.bass as bass
import concourse.tile as tile
from concourse import bacc, mybir
from concourse.bass_utils import run_bass_kernel_spmd

N_CORES = 8
B = 32                      # number of graphs
P = 128                     # partitions
DH = 128                    # feature dims (DA == DR == DH == 128)

_kernel_cache: dict = {}


def _col_chunks(n):
    """Split n columns into matmul chunks of <=512 that never cross a
    512-element PSUM bank boundary."""
    out, i = [], 0
    while i < n:
        w = min(512, n - i)
        out.append((i, w))
        i += w
    return out


def _layout(slot_shapes):
    """Packed input/output column layouts shared by builder and packer."""
    G = len(slot_shapes)
    o_res, o_atom, o_out = [], [], []
    nRc = sum(s[1] for s in slot_shapes)
    cur = 2 * DH + nRc                      # [mT | wvT | valid | slots...]
    for ac, rc, aw in slot_shapes:
        o_res.append(cur)
        cur += rc * P
        o_atom.append(cur)
        cur += aw
    W_in = cur
    cur = 0
    for ac, rc, aw in slot_shapes:
        o_out.append(cur)                   # [U^T slot | den slot] per slot
        cur += aw + ac
    W_out = cur
    return o_res, o_atom, o_out, W_in, W_out


def _build_kernel(slot_shapes):
    """One SPMD program: slots with per-rank (atom-chunks, residue-chunks,
    atom-width) capacities given by slot_shapes = ((AC, RC, AW), ...)."""
    G = len(slot_shapes)
    nAc = sum(s[0] for s in slot_shapes)
    nRc = sum(s[1] for s in slot_shapes)
    r_offs = np.concatenate([[0], np.cumsum([s[1] for s in slot_shapes])])
    a_offs = np.concatenate([[0], np.cumsum([s[0] for s in slot_shapes])])
    max_aw = max(s[2] for s in slot_shapes)
    o_res, o_atom, o_out, W_in, W_out = _layout(slot_shapes)
    o_valid = 2 * DH

    f32 = mybir.dt.float32
    bf16 = mybir.dt.bfloat16

    nc = bacc.Bacc("TRN2")
    inp = nc.dram_tensor("inp", [P, W_in], bf16, kind="ExternalInput")
    outp = nc.dram_tensor("outp", [P, W_out], bf16, kind="ExternalOutput")

    with tile.TileContext(nc) as tc:
        with (
            tc.tile_pool(name="singles", bufs=1) as singles,
            tc.tile_pool(name="ps_s", bufs=2, space="PSUM") as ps_s,
            tc.tile_pool(name="ps_u", bufs=1, space="PSUM") as ps_u,
            tc.tile_pool(name="ps_kv", bufs=2, space="PSUM") as ps_kv,
        ):
            IN_sb = singles.tile([P, W_in], bf16)
            CT_sb = singles.tile([P, nRc * P], bf16)
            V_sb = singles.tile([P, nRc, DH], bf16)
            ES_sb = singles.tile([P, nRc, max_aw], bf16)
            OUT_sb = singles.tile([P, W_out], bf16)
            warm_sb = singles.tile([P, 8], f32)

            # warm up the ACT exp table while DMAs run
            nc.gpsimd.memset(warm_sb[:], 0.0)
            nc.scalar.activation(
                warm_sb[:, 0:1], warm_sb[:, 1:2],
                mybir.ActivationFunctionType.Exp,
            )
            # warm up the PE p-state (2.4GHz needs ~3us of continuous busy);
            # few enough that they finish before slot-0 data lands
            wsrc = singles.tile([P, 512], bf16)
            nc.vector.memset(wsrc[:], 0.0)
            for _ in range(4):
                pw = ps_s.tile([P, 1024], f32, tag="s")
                nc.tensor.matmul(
                    pw[:, :512], wsrc[:, :DH], wsrc[:], start=True, stop=True
                )

            # input DMAs: [aux | res0] first (unblocks CT0), then atom0,
            # then one per remaining slot
            nc.sync.dma_start(IN_sb[:, : o_atom[0]], inp[:, : o_atom[0]])
            nc.sync.dma_start(
                IN_sb[:, o_atom[0] : o_atom[0] + slot_shapes[0][2]],
                inp[:, o_atom[0] : o_atom[0] + slot_shapes[0][2]],
            )
            for g in range(1, G):
                lo, hi = o_res[g], o_atom[g] + slot_shapes[g][2]
                nc.sync.dma_start(IN_sb[:, lo:hi], inp[:, lo:hi])

            mT_ap = IN_sb[:, :DH]
            wvT_ap = IN_sb[:, DH : 2 * DH]

            def emit_ctv(g):
                """CT and V for slot g's residues."""
                ac, rc, aw = slot_shapes[g]
                if rc == 0:
                    return
                r0 = r_offs[g] * P            # CT/V-space offset
                ri = o_res[g]                 # IN_sb offset
                rcols = rc * P
                pc = ps_kv.tile([P, 512], f32, tag="kv")
                nc.tensor.matmul(
                    pc[:, :rcols], mT_ap, IN_sb[:, ri : ri + rcols],
                    start=True, stop=True,
                )
                if g == 0:
                    # split so S(0, k=0) can start after just the first chunk
                    nc.scalar.copy(CT_sb[:, r0 : r0 + P], pc[:, :P])
                    nc.vector.tensor_copy(
                        CT_sb[:, r0 + P : r0 + rcols], pc[:, P:rcols]
                    )
                else:
                    nc.vector.tensor_copy(CT_sb[:, r0 : r0 + rcols], pc[:, :rcols])
                pv = ps_kv.tile([P, 512], f32, tag="kv")
                for k in range(rc):
                    nc.tensor.matmul(
                        pv[:, k * P : (k + 1) * P],
                        IN_sb[:, ri + k * P : ri + (k + 1) * P],
                        wvT_ap,
                        start=True, stop=True,
                    )
                nc.vector.tensor_copy(
                    V_sb[:, r_offs[g] : r_offs[g] + rc, :], pv[:, :rcols]
                )

            def emit_scores(g):
                """S^T chunks + exp for slot g."""
                ac, rc, aw = slot_shapes[g]
                if rc == 0 or aw == 0:
                    return
                ai = o_atom[g]
                for k in range(rc):
                    kg = r_offs[g] + k
                    ps = ps_s.tile([P, 1024], f32, tag="s")
                    for c, w in _col_chunks(aw):
                        nc.tensor.matmul(
                            ps[:, c : c + w],
                            CT_sb[:, kg * P : (kg + 1) * P],
                            IN_sb[:, ai + c : ai + c + w],
                            start=True, stop=True,
                        )
                    nc.scalar.activation(
                        ES_sb[:, kg, :aw], ps[:, :aw],
                        mybir.ActivationFunctionType.Exp,
                    )

            def emit_u(g):
                """U^T accumulation + denominator + output copy for slot g."""
                ac, rc, aw = slot_shapes[g]
                if aw == 0 or rc == 0:
                    return
                oo = o_out[g]
                # the last slot uses the (long-idle) kv pool so its U never
                # waits on the previous slot's output copy, and its copies
                # run on ACT (done with exp) + DVE in parallel
                tail = g == G - 1 and aw <= 512
                if tail:
                    pu = ps_kv.tile([P, 512], f32, tag="kv")
                    pden = ps_kv.tile([P, 512], f32, tag="kv")
                else:
                    pu = ps_u.tile([P, 1024], f32, tag="u")
                    pden = pu
                dcol = 0 if tail else aw
                for c, w in _col_chunks(aw):
                    for k in range(rc):
                        kg = r_offs[g] + k
                        nc.tensor.matmul(
                            pu[:, c : c + w],
                            V_sb[:, kg, :],
                            ES_sb[:, kg, c : c + w],
                            start=(k == 0), stop=(k == rc - 1),
                        )
                # denominator: single-column matmuls into spare PSUM columns
                for t in range(ac):
                    tw = min(P, aw - t * P)
                    if tw <= 0:
                        break
                    for k in range(rc):
                        kg = r_offs[g] + k
                        nc.tensor.matmul(
                            pden[:tw, dcol + t : dcol + t + 1],
                            ES_sb[:, kg, t * P : t * P + tw],
                            IN_sb[:, o_valid + kg : o_valid + kg + 1],
                            start=(k == 0), stop=(k == rc - 1),
                            skip_group_check=True,
                        )
                if tail:
                    nc.scalar.copy(OUT_sb[:, oo : oo + aw], pu[:, :aw])
                    nc.vector.tensor_copy(
                        OUT_sb[:, oo + aw : oo + aw + ac], pden[:, :ac]
                    )
                else:
                    nc.vector.tensor_copy(
                        OUT_sb[:, oo : oo + aw + ac], pu[:, : aw + ac]
                    )
                nc.sync.dma_start(
                    outp[:, oo : oo + aw + ac], OUT_sb[:, oo : oo + aw + ac]
                )

            for g in range(min(2, G)):
                emit_ctv(g)
            emit_scores(0)
            for g in range(2, G):
                emit_ctv(g)
            if G > 1:
                emit_scores(1)
            for g in range(G):
                if g + 2 < G:
                    emit_scores(g + 2)
                emit_u(g)

    nc.compile()
    return nc


def kernel(atom_h, residue_h, atom_batch, residue_batch, W_q, W_k, W_v):
    atom_h = np.asarray(atom_h, dtype=np.float32)
    residue_h = np.asarray(residue_h, dtype=np.float32)
    atom_batch = np.asarray(atom_batch)
    residue_batch = np.asarray(residue_batch)
    W_q = np.asarray(W_q, dtype=np.float32)
    W_k = np.asarray(W_k, dtype=np.float32)
    W_v = np.asarray(W_v, dtype=np.float32)
    bf = ml_dtypes.bfloat16

    A = atom_h.shape[0]
    R = residue_h.shape[0]
    n_b = max(B, int(atom_batch.max()) + 1 if A else B,
              int(residue_batch.max()) + 1 if R else B)

    ac = np.bincount(atom_batch, minlength=n_b)
    rc = np.bincount(residue_batch, minlength=n_b)
    a_off = np.concatenate([[0], np.cumsum(ac)])
    r_off = np.concatenate([[0], np.cumsum(rc)])
    a_ch = np.maximum(1, -(-ac // P))          # atom chunks per graph
    r_ch = np.maximum(1, -(-rc // P))          # residue chunks per graph

    G = (n_b + N_CORES - 1) // N_CORES          # slots per core

    # assign graphs to (rank, core): try a few sort keys, keep the cheapest
    best = None
    for key in (r_ch * 10000 + ac, a_ch * 100000 + r_ch * 10 + ac // 100,
                r_ch * a_ch * 10000 + ac):
        order = np.argsort(-key, kind="stable")
        shapes, cost = [], 0
        for i in range(G):
            grp = order[i * N_CORES : (i + 1) * N_CORES]
            AC = int(a_ch[grp].max()) if len(grp) else 1
            RC = int(r_ch[grp].max()) if len(grp) else 1
            AW = int(-(-int(ac[grp].max()) // 8) * 8) if len(grp) else 8
            AW = max(AW, 8)
            shapes.append((AC, RC, AW))
            cost += RC * AW
        if best is None or cost < best[0]:
            best = (cost, shapes, order)
    cost, shapes, order = best
    # permute ranks so the cheapest (RC, AW) rank runs last (short tail)
    perm = sorted(range(G), key=lambda i: (shapes[i][1], shapes[i][2]),
                  reverse=True)
    slot_shapes = tuple(shapes[i] for i in perm)
    order = np.concatenate(
        [order[i * N_CORES : (i + 1) * N_CORES] for i in perm]
    ) if len(order) >= G * N_CORES else order

    nAc = sum(s[0] for s in slot_shapes)
    nRc = sum(s[1] for s in slot_shapes)
    a_offs = np.concatenate([[0], np.cumsum([s[0] for s in slot_shapes])])
    r_offs = np.concatenate([[0], np.cumsum([s[1] for s in slot_shapes])])
    o_res, o_atom, o_out, W_in, W_out = _layout(slot_shapes)
    o_valid = 2 * DH

    key = slot_shapes
    if key not in _kernel_cache:
        _kernel_cache[key] = _build_kernel(slot_shapes)
    nc = _kernel_cache[key]

    scale = 1.0 / np.sqrt(np.float32(DH))
    mT = ((W_q.T @ W_k) * scale).T.astype(bf)   # lhsT for CT = M' @ R^T
    wvT = np.ascontiguousarray(W_v.T).astype(bf)

    in_maps = []
    for c in range(N_CORES):
        inp_c = np.zeros((P, W_in), dtype=bf)
        inp_c[:, :DH] = mT
        inp_c[:, DH : 2 * DH] = wvT
        for i in range(G):
            idx = i * N_CORES + c
            if idx >= len(order):
                continue
            g = order[idx]
            na, nr = int(ac[g]), int(rc[g])
            if na:
                inp_c[:, o_atom[i] : o_atom[i] + na] = (
                    atom_h[a_off[g] : a_off[g] + na].T.astype(bf)
                )
            if nr:
                inp_c[:, o_res[i] : o_res[i] + nr] = (
                    residue_h[r_off[g] : r_off[g] + nr].T.astype(bf)
                )
                flat = np.zeros(slot_shapes[i][1] * P, dtype=bf)
                flat[:nr] = 1.0
                inp_c[:, o_valid + r_offs[i] : o_valid + r_offs[i + 1]] = (
                    flat.reshape(-1, P).T
                )
        in_maps.append({"inp": inp_c})

    res = run_bass_kernel_spmd(nc, in_maps, core_ids=list(range(N_CORES)))

    result = atom_h.copy()
    for c in range(N_CORES):
        full = np.asarray(res.results[c]["outp"], dtype=np.float32)  # [128, W_out]
        for i in range(G):
            idx = i * N_CORES + c
            if idx >= len(order):
                continue
            g = order[idx]
            na, nr = int(ac[g]), int(rc[g])
            if na == 0 or nr == 0:
                continue
            AC_i, _, aw_i = slot_shapes[i]
            cols = slice(o_out[i], o_out[i] + na)
            dn = full[:, o_out[i] + aw_i : o_out[i] + aw_i + AC_i]
            d = dn.T.reshape(-1)[:na]
            d = np.where(d > 0, d, 1.0)
            result[a_off[g] : a_off[g] + na] += (full[:, cols] / d[None, :]).T
    return result
